# revision 1
# baseline (speedup 1.0000x reference)
"""AttentiveFP forward pass as a Bass/Tile kernel on 8 Trainium2 NeuronCores.

Strategy: data-parallel by graph blocks (256 graphs/core); edges assigned to
the core owning their dst node (edges freely cross cores); per-core windowed
segment-softmax aggregation via selection-matrix matmuls on the PE; node
features kept transposed on-chip so GATv2+GRU elementwise runs with
per-feature biases as per-partition ACT biases; node feature tables exchanged
between layers with AllGather collectives; per-edge source rows fetched with
indirect DMA gathers.
"""
import sys, os
sys.path.insert(0, '/opt/trn_rl_repo')
import numpy as np
from contextlib import ExitStack

import concourse.bass as bass
import concourse.mybir as mybir
import concourse.tile as tile
from concourse.bass import IndirectOffsetOnAxis
from concourse.mybir import AluOpType as alu, ActivationFunctionType as act

G_DEFAULT = 2048


def preprocess(edge_index, batch, n_cores=8, G=2048, CW=5):
    src = np.asarray(edge_index[0]).astype(np.int64)
    dst = np.asarray(edge_index[1]).astype(np.int64)
    batch = np.asarray(batch).astype(np.int64)
    N = batch.shape[0]
    GPC = G // n_cores
    gstart = np.searchsorted(batch, np.arange(0, G + 1, GPC))
    ncounts = np.diff(gstart)
    NLOC = int(np.ceil(ncounts.max() / 128) * 128)
    NWIN = NLOC // 128
    NCH = NWIN * CW

    node_owner = np.searchsorted(gstart, np.arange(N), side='right') - 1
    ag_row = (node_owner * NLOC + (np.arange(N) - gstart[node_owner])).astype(np.int64)
    owner = node_owner[dst]

    cores = []
    for c in range(n_cores):
        ns, ne = int(gstart[c]), int(gstart[c + 1])
        nn = ne - ns
        m = owner == c
        eidx = np.nonzero(m)[0]
        dl = dst[eidx] - ns
        order = np.argsort(dl, kind='stable')
        eidx = eidx[order]; dl = dl[order]
        win = dl // 128
        counts = np.bincount(win, minlength=NWIN)
        assert counts.max() <= CW * 128, f"window overflow {counts.max()}"
        pos = np.concatenate([[0], np.cumsum(counts)])[:-1]
        within = np.arange(len(dl)) - pos[win]
        slots = (win * CW * 128 + within).astype(np.int64)

        sl_src_ag = np.zeros(NCH * 128, np.int32)          # gather row in ag table
        sl_dstloc = np.full(NCH * 128, -1.0, np.float32)   # dst within window, -1 pad
        sl_edge = np.zeros(NCH * 128, np.int64)            # original edge id
        sl_fill = np.zeros(NCH * 128, bool)
        sl_src_ag[slots] = ag_row[src[eidx]]
        sl_dstloc[slots] = (dl % 128).astype(np.float32)
        sl_edge[slots] = eidx
        sl_fill[slots] = True

        # device-layout meta, per window loads:
        # srcblk [NWIN, 128, CW] int32 ; dstlocblk [NWIN, 128, CW] f32 ;
        # dstrowblk [NWIN, 1, CW*128] f32
        srcblk = sl_src_ag.reshape(NWIN, CW, 128).transpose(0, 2, 1).copy()
        dstlocblk = sl_dstloc.reshape(NWIN, CW, 128).transpose(0, 2, 1).copy()
        dstrowblk = sl_dstloc.reshape(NWIN, 1, CW * 128).copy()

        # mol phase: node chunk k -> graphs gloc (local graph id 0..GPC-1), pad -1
        gloc = np.full(NLOC, -1.0, np.float32)
        gloc[:nn] = (batch[ns:ne] - c * GPC).astype(np.float32)
        glocblk = gloc.reshape(NWIN, 128, 1).copy()

        cores.append(dict(ns=ns, ne=ne, nn=nn,
                          srcblk=srcblk, dstlocblk=dstlocblk, dstrowblk=dstrowblk,
                          glocblk=glocblk, sl_edge=sl_edge, sl_fill=sl_fill))
    return dict(cores=cores, gstart=gstart, NLOC=NLOC, NWIN=NWIN, NCH=NCH, CW=CW,
                GPC=GPC, n_cores=n_cores)

# ---------------- walrus sync-wait splitting ----------------
MAX_WAITS = 1

def split_waits(nc):
    eng_map = nc.engines
    for bbname, bassbb in nc.bb_map.items():
        insts = bassbb.bb.instructions
        i = 0
        while i < len(insts):
            inst = insts[i]
            si = inst.sync_info
            if si is not None and si.on_wait is not None and len(si.on_wait) > MAX_WAITS:
                waits = list(si.on_wait)
                si.on_wait = waits[-MAX_WAITS:]
                rest = waits[:-MAX_WAITS]
                for j in range(0, len(rest), MAX_WAITS):
                    eng = eng_map[inst.engine]
                    nop = eng.nop(nofuse=True)
                    nop_inst = nop.ins
                    for obb in nc.bb_map.values():
                        lst = obb.bb.instructions
                        for k in range(len(lst) - 1, -1, -1):
                            if lst[k].name == nop_inst.name:
                                del lst[k]
                                break
                    nsi = nop_inst.sync_info
                    chunk = rest[j:j + MAX_WAITS]
                    if nsi is None:
                        nop_inst.sync_info = mybir.SyncInfo(on_wait=chunk, on_update=[])
                    else:
                        nsi.on_wait = chunk
                    insts.insert(i, nop_inst)
                    i += 1
            i += 1


class TileContextFixed(tile.TileContext):
    def __exit__(self, *args):
        r = super().__exit__(*args)
        split_waits(self.nc)
        return r


F32 = mybir.dt.float32

F32 = mybir.dt.float32
I32 = mybir.dt.int32
EPS = 1e-30


def wpack_layout():
    """Returns (layout dict name->(off, cols), total_cols). All blocks [128, cols]."""
    L = {}
    off = 0
    def add(name, cols):
        nonlocal off
        L[name] = (off, cols)
        off += cols
    add("iota_sq", 128)
    add("iota256", 256)
    add("iota_col", 1)
    add("ones_col", 1)
    add("attl_sq", 256)      # g_att_l replicated rows
    add("attm_sq", 256)      # mol_att replicated rows
    for l in range(3):
        add(f"att{l}_sq", 256)
    add("W1T", 2 * 256)      # u = x @ W1.T : rhs chunks [128,256] x2
    for i in range(4):       # 0..2 atom, 3 mol
        add(f"WlTr{i}", 2 * 256)
        add(f"WrTr{i}", 2 * 256)
        for k in range(2):
            for b in range(2):
                add(f"WrTl{i}_{k}{b}", 128)
    for k in range(2):
        for b in range(2):
            add(f"gl2T_{k}{b}", 128)
    add("gb2", 2)            # g_bias2 cols x2 blocks
    add("attr_col", 2)       # g_att_r as 2 col chunks [128,1]
    for g in range(5):       # gru0, agru0..2, mgru
        for j in range(12):
            for b in range(2):
                add(f"gru{g}_w{j}{b}", 128)
        for j in range(4):   # br, bz, bin, bhn
            for b in range(2):
                add(f"gru{g}_b{j}{b}", 1)
    for i in range(3):
        add(f"ab{i}", 2)     # atom bias cols x2
    add("molb", 2)
    add("id0", 256)          # [I128 | 0]
    add("id1", 256)          # [0 | I128]
    add("w1T", 2 * 128)      # mlp w1.T chunks
    add("b1", 1)
    add("w2T", 64)
    add("b2_", 1)
    return L, off


def make_wpack(inp):
    """Host: build wpack [128, WCOLS] f32 from the model inputs dict."""
    L, total = wpack_layout()
    W = np.zeros((128, total), np.float32)
    def put(name, arr):
        off, cols = L[name]
        assert arr.shape == (128, cols), (name, arr.shape, cols)
        W[:, off:off + cols] = arr
    put("iota_sq", np.tile(np.arange(128, dtype=np.float32), (128, 1)))
    put("iota256", np.tile(np.arange(256, dtype=np.float32), (128, 1)))
    put("iota_col", np.arange(128, dtype=np.float32).reshape(128, 1))
    put("ones_col", np.ones((128, 1), np.float32))
    put("attl_sq", np.tile(inp['g_att_l'], (128, 1)))
    put("attm_sq", np.tile(inp['mol_att'], (128, 1)))
    for l in range(3):
        put(f"att{l}_sq", np.tile(inp['atom_att'][l], (128, 1)))
    W1 = inp['g_lin1_w'][:, :256]
    W1T = W1.T.astype(np.float32)                      # [256 k, 256 h']
    put("W1T", np.concatenate([W1T[0:128], W1T[128:256]], axis=1))
    Wls = [inp['atom_Wl'][0], inp['atom_Wl'][1], inp['atom_Wl'][2], inp['mol_Wl']]
    Wrs = [inp['atom_Wr'][0], inp['atom_Wr'][1], inp['atom_Wr'][2], inp['mol_Wr']]
    for i in range(4):
        WT = Wls[i].T.astype(np.float32)
        put(f"WlTr{i}", np.concatenate([WT[0:128], WT[128:256]], axis=1))
        WT = Wrs[i].T.astype(np.float32)
        put(f"WrTr{i}", np.concatenate([WT[0:128], WT[128:256]], axis=1))
        for k in range(2):
            for b in range(2):
                put(f"WrTl{i}_{k}{b}", WT[k * 128:(k + 1) * 128, b * 128:(b + 1) * 128])
    g2T = inp['g_lin2_w'].T.astype(np.float32)         # [h k, h' m]
    for k in range(2):
        for b in range(2):
            put(f"gl2T_{k}{b}", g2T[k * 128:(k + 1) * 128, b * 128:(b + 1) * 128])
    put("gb2", inp['g_bias'].reshape(2, 128).T.astype(np.float32))
    put("attr_col", inp['g_att_r'].reshape(2, 128).T.astype(np.float32))
    grus = [('gru0_wih', 'gru0_whh', 'gru0_bih', 'gru0_bhh', None),
            ('agru_wih', 'agru_whh', 'agru_bih', 'agru_bhh', 0),
            ('agru_wih', 'agru_whh', 'agru_bih', 'agru_bhh', 1),
            ('agru_wih', 'agru_whh', 'agru_bih', 'agru_bhh', 2),
            ('mgru_wih', 'mgru_whh', 'mgru_bih', 'mgru_bhh', None)]
    for g, (wi, wh, bi, bh, l) in enumerate(grus):
        wih = inp[wi] if l is None else inp[wi][l]     # [768, 256]
        whh = inp[wh] if l is None else inp[wh][l]
        bih = inp[bi] if l is None else inp[bi][l]
        bhh = inp[bh] if l is None else inp[bh][l]
        # gates rows: r 0:256, z 256:512, n 512:768
        wihT = wih.T.astype(np.float32)                # [256 k, 768]
        whhT = whh.T.astype(np.float32)
        # j layout: r: 0,1 h-side kchunks; 2,3 x-side; z: 4..7; inn(h): 8,9; hn(x): 10,11
        for k in range(2):
            for b in range(2):
                ks, bs = slice(k * 128, (k + 1) * 128), slice(b * 128, (b + 1) * 128)
                put(f"gru{g}_w{0 + k}{b}", wihT[ks, 0:256][:, bs])
                put(f"gru{g}_w{2 + k}{b}", whhT[ks, 0:256][:, bs])
                put(f"gru{g}_w{4 + k}{b}", wihT[ks, 256:512][:, bs])
                put(f"gru{g}_w{6 + k}{b}", whhT[ks, 256:512][:, bs])
                put(f"gru{g}_w{8 + k}{b}", wihT[ks, 512:768][:, bs])
                put(f"gru{g}_w{10 + k}{b}", whhT[ks, 512:768][:, bs])
        br = (bih[0:256] + bhh[0:256]).reshape(2, 128).T
        bz = (bih[256:512] + bhh[256:512]).reshape(2, 128).T
        bin_ = bih[512:768].reshape(2, 128).T
        bhn = bhh[512:768].reshape(2, 128).T
        for j, arr in enumerate([br, bz, bin_, bhn]):
            for b in range(2):
                put(f"gru{g}_b{j}{b}", arr[:, b:b + 1].astype(np.float32))
    for i in range(3):
        put(f"ab{i}", inp['atom_bias'][i].reshape(2, 128).T.astype(np.float32))
    put("molb", inp['mol_bias'].reshape(2, 128).T.astype(np.float32))
    I = np.eye(128, dtype=np.float32)
    put("id0", np.concatenate([I, np.zeros((128, 128), np.float32)], 1))
    put("id1", np.concatenate([np.zeros((128, 128), np.float32), I], 1))
    w1T = inp['mlp_w1'].T.astype(np.float32)           # [256, 128]
    put("w1T", np.concatenate([w1T[0:128], w1T[128:256]], 1))
    put("b1", inp['mlp_b1'].reshape(128, 1).astype(np.float32))
    put("w2T", inp['mlp_w2'].T.astype(np.float32))     # [128, 64]
    put("b2_", np.zeros((128, 1), np.float32) + np.pad(inp['mlp_b2'], (0, 64)).reshape(128, 1))
    return W


def build_kernel(NLOC, NWIN, CW, NG, n_cores, taps=(), dt_tab=F32, stop_after=None):
    H = 256
    NCH = NWIN * CW
    assert NG in (128, 256)
    NGB = NG // 128

    nc = bass.Bass(num_devices=n_cores)
    L, WCOLS = wpack_layout()

    def dram_in(name, shape, dt=F32):
        return nc.dram_tensor(name, list(shape), dt, kind="ExternalInput")

    xinT = dram_in("xinT", [65, NLOC])
    srcblk = dram_in("srcblk", [NWIN, 128, CW], I32)
    dstlocblk = dram_in("dstlocblk", [NWIN, 128, CW])
    dstrowblk = dram_in("dstrowblk", [NWIN, 1, CW * 128])
    glocblk = dram_in("glocblk", [NWIN, 128, 1])
    eaTd = dram_in("eaT", [NCH, 16, 128])
    wpack = dram_in("wpack", [128, WCOLS])
    lin1Td = dram_in("lin1T", [65, 256])
    W2Td = dram_in("W2T", [16, 256])
    w3Td = dram_in("w3T", [65, 1])

    y = nc.dram_tensor("y", [1, 256], F32, kind="ExternalOutput")

    xT_a = nc.dram_tensor("xT_a", [2, 128, NLOC], F32)
    xT_b = nc.dram_tensor("xT_b", [2, 128, NLOC], F32)
    w_c = nc.dram_tensor("w_c", [NWIN, 128, 1], F32)
    cc_in = nc.dram_tensor("cc_in", [NLOC, H], dt_tab)
    tab_full = nc.dram_tensor("tab_full", [n_cores * NLOC, H], dt_tab, addr_space="Shared")
    hr_row = nc.dram_tensor("hr_row", [NLOC, H], dt_tab)
    x_row = nc.dram_tensor("x_row", [NLOC, H], dt_tab)
    hl_md = nc.dram_tensor("hl_m", [NLOC, H], dt_tab)

    dbg = {}
    for t in taps:
        shp = [3, 128, NLOC] if t.startswith('ags') else [2, 128, NLOC]
        dbg[t] = nc.dram_tensor(f"dbg_{t}", shp, F32, kind="ExternalOutput")

    with TileContextFixed(nc) as tc, ExitStack() as ctx:
        wpool = ctx.enter_context(tc.tile_pool(name="weights", bufs=1))
        cpool = ctx.enter_context(tc.tile_pool(name="chunk", bufs=2))
        spool = ctx.enter_context(tc.tile_pool(name="small", bufs=3))
        npool = ctx.enter_context(tc.tile_pool(name="node", bufs=2))
        gpool = ctx.enter_context(tc.tile_pool(name="grup", bufs=1))
        molpool = ctx.enter_context(tc.tile_pool(name="molp", bufs=1))
        mpool = ctx.enter_context(tc.tile_pool(name="meta", bufs=2))
        pp_chunk = ctx.enter_context(tc.tile_pool(name="pschunk", bufs=2, space="PSUM"))
        pp_acc = ctx.enter_context(tc.tile_pool(name="psacc", bufs=1, space="PSUM"))
        pp_gru = ctx.enter_context(tc.tile_pool(name="psgru", bufs=2, space="PSUM"))
        pp_misc = ctx.enter_context(tc.tile_pool(name="psmisc", bufs=1, space="PSUM"))

        wp = wpool.tile([128, WCOLS], F32, tag="wp")
        nc.sync.dma_start(wp[:], wpack.ap())
        def W(name):
            off, cols = L[name]
            return wp[:, off:off + cols]
        iota_sq, iota256 = W("iota_sq"), W("iota256")
        ones_col = W("ones_col")
        ident = W("id0")[:, 0:128]
        ones1 = wpool.tile([1, 128], F32, tag="ones1")
        nc.vector.memset(ones1[:], 1.0)
        lin1T = wpool.tile([65, 256], F32, tag="lin1T")
        nc.sync.dma_start(lin1T[:], lin1Td.ap())
        W2T = wpool.tile([16, 256], F32, tag="W2T")
        nc.sync.dma_start(W2T[:], W2Td.ap())
        w3T = wpool.tile([65, 1], F32, tag="w3T")
        nc.sync.dma_start(w3T[:], w3Td.ap())

        def misc_ps(cols=512):
            return pp_misc.tile([128, cols], F32, tag="misc", name="miscps")

        def tap(name, xT_cur):
            if name in dbg:
                tt = npool.tile([128, 256], F32, tag="tapt")
                for b in range(2):
                    for w in range(NWIN):
                        sl = slice(w * 128, (w + 1) * 128)
                        nc.sync.dma_start(tt[:, 0:128], xT_cur.ap()[b][:, sl])
                        nc.sync.dma_start(dbg[name].ap()[b][:, sl], tt[:, 0:128])

        def elu(out_ap, x_tile, pool, wcols):
            u = pool.tile([128, wcols], F32, tag=f"elu_u{wcols}", name="eluu")
            nc.vector.tensor_scalar(out=u[:], in0=x_tile[:], scalar1=0.0,
                                    scalar2=None, op0=alu.min)
            eu = pool.tile([128, wcols], F32, tag=f"elu_e{wcols}", name="elue")
            nc.scalar.activation(eu[:], u[:], act.Exp)
            t = pool.tile([128, wcols], F32, tag=f"elu_t{wcols}", name="elut")
            nc.vector.scalar_tensor_tensor(out=t[:], in0=u[:], scalar=-1.0, in1=eu[:],
                                           op0=alu.mult, op1=alu.add)
            nc.vector.scalar_tensor_tensor(out=out_ap, in0=x_tile[:], scalar=-1.0,
                                           in1=t[:], op0=alu.add, op1=alu.add)

        def gru(g, hT, xT, n, WIDE, relu_out=True):
            """transposed gru: hT/xT [128, 2*WIDE]; returns xn [128, 2*WIDE]."""
            xn = gpool.tile([128, 2 * WIDE], F32, tag=f"gru_xn{WIDE}", name="gruxn")
            for b in range(2):
                bs = slice(b * WIDE, (b + 1) * WIDE)
                def gate_mm(ps, joff_h, joff_x):
                    for k in range(2):
                        ks = slice(k * WIDE, (k + 1) * WIDE)
                        if joff_h is not None:
                            nc.tensor.matmul(ps[:], lhsT=W(f"gru{g}_w{joff_h + k}{b}"),
                                             rhs=hT[:, ks], start=(k == 0),
                                             stop=(k == 1 and joff_x is None))
                        if joff_x is not None:
                            nc.tensor.matmul(ps[:], lhsT=W(f"gru{g}_w{joff_x + k}{b}"),
                                             rhs=xT[:, ks],
                                             start=(k == 0 and joff_h is None),
                                             stop=(k == 1))
                rps = pp_gru.tile([128, WIDE], F32, tag="gp", name="rps")
                gate_mm(rps, 0, 2)
                r = gpool.tile([128, WIDE], F32, tag=f"gru_r{WIDE}", name="grur")
                nc.scalar.activation(r[:], rps[:], act.Sigmoid, bias=W(f"gru{g}_b0{b}"))
                zps = pp_gru.tile([128, WIDE], F32, tag="gp", name="zps")
                gate_mm(zps, 4, 6)
                z = gpool.tile([128, WIDE], F32, tag=f"gru_z{WIDE}", name="gruz")
                nc.scalar.activation(z[:], zps[:], act.Sigmoid, bias=W(f"gru{g}_b1{b}"))
                ips = pp_gru.tile([128, WIDE], F32, tag="gp", name="ips")
                gate_mm(ips, 8, None)
                hps = pp_gru.tile([128, WIDE], F32, tag="gp", name="hps")
                gate_mm(hps, None, 10)
                t1 = gpool.tile([128, WIDE], F32, tag=f"gru_t1{WIDE}", name="grut1")
                nc.vector.scalar_tensor_tensor(out=t1[:], in0=hps[:],
                                               scalar=W(f"gru{g}_b3{b}"), in1=r[:],
                                               op0=alu.add, op1=alu.mult)
                t2 = gpool.tile([128, WIDE], F32, tag=f"gru_t2{WIDE}", name="grut2")
                nc.vector.tensor_tensor(out=t2[:], in0=t1[:], in1=ips[:], op=alu.add)
                nn_ = gpool.tile([128, WIDE], F32, tag=f"gru_n{WIDE}", name="grun")
                nc.scalar.activation(nn_[:], t2[:], act.Tanh, bias=W(f"gru{g}_b2{b}"))
                d = gpool.tile([128, WIDE], F32, tag=f"gru_d{WIDE}", name="grud")
                nc.vector.tensor_tensor(out=d[:], in0=xT[:, bs], in1=nn_[:], op=alu.subtract)
                zd = gpool.tile([128, WIDE], F32, tag=f"gru_zd{WIDE}", name="gruzd")
                nc.vector.tensor_tensor(out=zd[:], in0=z[:], in1=d[:], op=alu.mult)
                if relu_out:
                    t3 = gpool.tile([128, WIDE], F32, tag=f"gru_t3{WIDE}", name="grut3")
                    nc.vector.tensor_tensor(out=t3[:], in0=nn_[:], in1=zd[:], op=alu.add)
                    nc.scalar.activation(xn[:, bs], t3[:], act.Relu)
                else:
                    nc.vector.tensor_tensor(out=xn[:, bs], in0=nn_[:], in1=zd[:], op=alu.add)
            return xn

        # ================= P0: projection =================
        for w in range(NWIN):
            sl = slice(w * 128, (w + 1) * 128)
            xin_t = mpool.tile([65, 128], F32, tag="xin", bufs=1)
            nc.sync.dma_start(xin_t[:], xinT.ap()[:, sl])
            x0ps = misc_ps(256)
            for b in range(2):
                nc.tensor.matmul(x0ps[:, b * 128:(b + 1) * 128],
                                 lhsT=lin1T[:, b * 128:(b + 1) * 128],
                                 rhs=xin_t[:], start=True, stop=True)
            x0T = npool.tile([128, 256], F32, tag="x0T")
            for b in range(2):
                nc.scalar.activation(x0T[:, b * 128:(b + 1) * 128],
                                     x0ps[:, b * 128:(b + 1) * 128], act.Lrelu, alpha=0.01)
                nc.sync.dma_start(xT_a.ap()[b][:, sl], x0T[:, b * 128:(b + 1) * 128])
            ups = pp_acc.tile([128, 1536], F32, tag="aggps", name="ups")
            for b in range(2):
                nc.tensor.matmul(ups[:, 0:256], lhsT=x0T[:, b * 128:(b + 1) * 128],
                                 rhs=W(f"W1T")[:, b * 256:(b + 1) * 256],
                                 start=(b == 0), stop=(b == 1))
            for b in range(2):
                nc.tensor.matmul(ups[:, 512:513], lhsT=x0T[:, b * 128:(b + 1) * 128],
                                 rhs=W("attr_col")[:, b:b + 1],
                                 start=(b == 0), stop=(b == 1))
            u_sb = npool.tile([128, 257], dt_tab, tag="tabsb", name="tabsb")
            nc.vector.tensor_copy(u_sb[:, 0:256], ups[:, 0:256])
            nc.vector.tensor_copy(u_sb[:, 256:257], ups[:, 512:513])
            nc.sync.dma_start(cc_in.ap()[sl, :], u_sb[:, 0:256])
            nc.sync.dma_start(w_c.ap()[w], u_sb[:, 256:257])

        def allgather():
            if n_cores == 1:
                nc.sync.dma_start(tab_full.ap()[:, :], cc_in.ap()[:, :])
            else:
                nc.gpsimd.collective_compute(
                    "AllGather", alu.bypass,
                    replica_groups=[list(range(n_cores))],
                    ins=[cc_in.ap()], outs=[tab_full.ap()])
        allgather()

        # ================= edge layers =================
        def edge_layer(kind, l, xT_src, xT_dst, last=False):
            gru_i = 0 if kind == 'gate' else 1 + l
            attw = W("attl_sq") if kind == 'gate' else W(f"att{l}_sq")
            for w in range(NWIN):
                sl = slice(w * 128, (w + 1) * 128)
                srct = mpool.tile([128, CW], I32, tag="srct")
                nc.sync.dma_start(srct[:], srcblk.ap()[w])
                dlc = mpool.tile([128, CW], F32, tag="dlc")
                nc.sync.dma_start(dlc[:], dstlocblk.ap()[w])
                drow = mpool.tile([1, CW * 128], F32, tag="drow", bufs=1)
                nc.sync.dma_start(drow[:], dstrowblk.ap()[w])
                xTw = npool.tile([128, 256], F32, tag="xTw")
                for b in range(2):
                    nc.sync.dma_start(xTw[:, b * 128:(b + 1) * 128], xT_src.ap()[b][:, sl])
                if kind == 'atom':
                    hrw = npool.tile([128, H], dt_tab, tag="hrw")
                    nc.sync.dma_start(hrw[:], hr_row.ap()[sl, :])
                    hrT = npool.tile([128, 256], F32, tag="hrT")
                    hrTps = misc_ps(256)
                    for b in range(2):
                        for k in range(2):
                            nc.tensor.matmul(hrTps[:, b * 128:(b + 1) * 128],
                                             lhsT=W(f"WrTl{l}_{k}{b}"),
                                             rhs=xTw[:, k * 128:(k + 1) * 128],
                                             start=(k == 0), stop=(k == 1))
                    nc.vector.tensor_copy(hrT[:], hrTps[:])
                else:
                    wwin = spool.tile([128, 1], F32, tag="wwin")
                    nc.sync.dma_start(wwin[:], w_c.ap()[w])

                aggps = pp_acc.tile([128, 1536], F32, tag="aggps", name="aggps")
                AGG = [0, 512]  # col offset of agg block b (separate banks)
                SCOL, RCOL = 1024, 1152

                for ci in range(CW):
                    first, lastc = (ci == 0), (ci == CW - 1)
                    chps = pp_chunk.tile([128, 512], F32, tag="chps")
                    # dst_bcast [:,256:384]
                    nc.tensor.matmul(chps[:, 256:384], lhsT=ones1[:],
                                     rhs=drow[:, ci * 128:(ci + 1) * 128],
                                     start=True, stop=True)
                    selT = spool.tile([128, 128], F32, tag="selT")
                    nc.vector.tensor_scalar(out=selT[:], in0=chps[:, 256:384],
                                            scalar1=W("iota_col"), scalar2=None,
                                            op0=alu.is_equal)
                    g = cpool.tile([128, H], dt_tab, tag="gather", bufs=4)
                    nc.gpsimd.indirect_dma_start(
                        out=g[:], out_offset=None, in_=tab_full.ap(),
                        in_offset=IndirectOffsetOnAxis(ap=srct[:, ci:ci + 1], axis=0))
                    if kind == 'atom':
                        nc.tensor.matmul(chps[:, 0:256], lhsT=selT[:], rhs=hrw[:],
                                         start=True, stop=True)
                    else:
                        eat = mpool.tile([16, 128], F32, tag="eat", bufs=1)
                        nc.sync.dma_start(eat[:], eaTd.ap()[w * CW + ci])
                        nc.tensor.matmul(chps[:, 0:256], lhsT=eat[:], rhs=W2T[:],
                                         start=True, stop=True)
                        nc.tensor.matmul(chps[:, 384:385], lhsT=selT[:], rhs=wwin[:],
                                         start=True, stop=True)
                    t_t = cpool.tile([128, 256], F32, tag="t_t")
                    nc.vector.scalar_tensor_tensor(out=t_t[:], in0=chps[:, 0:256],
                                                   scalar=0.0, in1=g[:],
                                                   op0=alu.add, op1=alu.add)
                    tl = cpool.tile([128, 256], F32, tag="tl")
                    nc.scalar.activation(tl[:], t_t[:], act.Lrelu, alpha=0.01)
                    escr = cpool.tile([128, 256], F32, tag="escr")
                    ecol = spool.tile([128, 1], F32, tag="ecol")
                    nc.vector.tensor_tensor(out=escr[:], in0=tl[:], in1=attw, op=alu.mult)
                    nc.vector.reduce_sum(out=ecol[:], in_=escr[:], axis=mybir.AxisListType.X)
                    ex = spool.tile([128, 1], F32, tag="ex")
                    if kind == 'gate':
                        e2 = spool.tile([128, 1], F32, tag="e2")
                        nc.vector.scalar_tensor_tensor(out=e2[:], in0=chps[:, 384:385],
                                                       scalar=0.0, in1=ecol[:],
                                                       op0=alu.add, op1=alu.add)
                        el = spool.tile([128, 1], F32, tag="el")
                        nc.scalar.activation(el[:], e2[:], act.Lrelu, alpha=0.01)
                        nc.scalar.activation(ex[:], el[:], act.Exp)
                    else:
                        nc.scalar.activation(ex[:], ecol[:], act.Exp)
                    selw = spool.tile([128, 128], F32, tag="selw")
                    nc.vector.scalar_tensor_tensor(out=selw[:], in0=iota_sq,
                                                   scalar=dlc[:, ci:ci + 1],
                                                   op0=alu.is_equal, op1=alu.mult,
                                                   in1=ex[:].to_broadcast([128, 128]))
                    val = t_t if kind == 'atom' else tl
                    for b in range(2):
                        nc.tensor.matmul(aggps[:, AGG[b]:AGG[b] + 128],
                                         lhsT=val[:, b * 128:(b + 1) * 128],
                                         rhs=selw[:], start=first, stop=lastc)
                    nc.tensor.matmul(aggps[0:1, SCOL:SCOL + 128], lhsT=ones_col, rhs=selw[:],
                                     start=first, stop=lastc)

                # ---- epilogue ----
                sraw = spool.tile([1, 128], F32, tag="sraw")
                nc.vector.tensor_copy(sraw[:], aggps[0:1, SCOL:SCOL + 128])
                srow = spool.tile([1, 128], F32, tag="srow")
                nc.vector.tensor_scalar(out=srow[:], in0=sraw[:],
                                        scalar1=EPS, scalar2=None, op0=alu.max)
                rrow = spool.tile([1, 128], F32, tag="rrow")
                nc.vector.reciprocal(rrow[:], srow[:])
                nc.tensor.matmul(aggps[:, RCOL:RCOL + 128], lhsT=ones1[:], rhs=rrow[:],
                                 start=True, stop=True)
                rbc = npool.tile([128, 128], F32, tag="rbc")
                nc.vector.tensor_copy(rbc[:], aggps[:, RCOL:RCOL + 128])
                hT = npool.tile([128, 256], F32, tag="hT")
                if kind == 'atom':
                    sps = misc_ps(128)
                    nc.tensor.matmul(sps[:, 0:128], lhsT=ones1[:],
                                     rhs=sraw[:], start=True, stop=True)
                    sbc = npool.tile([128, 128], F32, tag="sbc")
                    nc.vector.tensor_copy(sbc[:], sps[:, 0:128])
                    for b in range(2):
                        bs = slice(b * 128, (b + 1) * 128)
                        t1 = npool.tile([128, 128], F32, tag="ep_t1")
                        nc.vector.scalar_tensor_tensor(out=t1[:], in0=hrT[:, bs],
                                                       scalar=-1.0, in1=sbc[:],
                                                       op0=alu.mult, op1=alu.mult)
                        nc.vector.tensor_tensor(out=t1[:], in0=t1[:],
                                                in1=aggps[:, AGG[b]:AGG[b] + 128],
                                                op=alu.add)
                        hpre = npool.tile([128, 128], F32, tag="ep_hpre")
                        nc.vector.tensor_tensor(out=hpre[:], in0=t1[:], in1=rbc[:],
                                                op=alu.mult)
                        nc.vector.tensor_scalar(out=hpre[:], in0=hpre[:],
                                                scalar1=W(f"ab{l}")[:, b:b + 1],
                                                scalar2=None, op0=alu.add)
                        if f'cor1_{kind}{l}' in dbg:
                            nc.sync.dma_start(dbg[f'cor1_{kind}{l}'].ap()[b][:, sl], t1[:])
                            nc.sync.dma_start(dbg[f'cor2_{kind}{l}'].ap()[b][:, sl], hpre[:])
                        elu(hT[:, bs], hpre, npool, 128)
                else:
                    aggn = npool.tile([128, 256], F32, tag="aggn")
                    for b in range(2):
                        bs = slice(b * 128, (b + 1) * 128)
                        nc.vector.tensor_tensor(out=aggn[:, bs],
                                                in0=aggps[:, AGG[b]:AGG[b] + 128],
                                                in1=rbc[:], op=alu.mult)
                    h0ps = misc_ps(256)
                    for b in range(2):
                        for k in range(2):
                            nc.tensor.matmul(h0ps[:, b * 128:(b + 1) * 128],
                                             lhsT=W(f"gl2T_{k}{b}"),
                                             rhs=aggn[:, k * 128:(k + 1) * 128],
                                             start=(k == 0), stop=(k == 1))
                    for b in range(2):
                        bs = slice(b * 128, (b + 1) * 128)
                        hpre = npool.tile([128, 128], F32, tag="ep_hpre")
                        nc.vector.tensor_scalar(out=hpre[:], in0=h0ps[:, bs],
                                                scalar1=W("gb2")[:, b:b + 1],
                                                scalar2=None, op0=alu.add)
                        elu(hT[:, bs], hpre, npool, 128)

                tkey = f'hrt_{kind}{l}'
                if tkey in dbg and kind == 'atom':
                    for b in range(2):
                        nc.sync.dma_start(dbg[tkey].ap()[b][:, sl], hrT[:, b * 128:(b + 1) * 128])
                hkey = f'h_{kind}{l}'
                if hkey in dbg:
                    for b in range(2):
                        nc.sync.dma_start(dbg[hkey].ap()[b][:, sl], hT[:, b * 128:(b + 1) * 128])
                akey = f'ags_{kind}{l}'
                if akey in dbg:
                    agt = npool.tile([128, 256], F32, tag="agt")
                    for b in range(2):
                        nc.vector.tensor_copy(agt[:, b * 128:(b + 1) * 128],
                                              aggps[:, AGG[b]:AGG[b] + 128])
                        nc.sync.dma_start(dbg[akey].ap()[b][:, sl], agt[:, b * 128:(b + 1) * 128])
                    nc.sync.dma_start(dbg[akey].ap()[2][0:1, sl], srow[:])
                xnT = gru(gru_i, hT, xTw, 256, 128)
                for b in range(2):
                    nc.sync.dma_start(xT_dst.ap()[b][:, sl], xnT[:, b * 128:(b + 1) * 128])

                if not last:
                    nl = (l + 1) if kind == 'atom' else 0
                    hlps = misc_ps(256)
                    for k in range(2):
                        nc.tensor.matmul(hlps[:, 0:256], lhsT=xnT[:, k * 128:(k + 1) * 128],
                                         rhs=W(f"WlTr{nl}")[:, k * 256:(k + 1) * 256],
                                         start=(k == 0), stop=(k == 1))
                    hlsb = npool.tile([128, H], dt_tab, tag="tabsb", name="tabsb")
                    nc.vector.tensor_copy(hlsb[:], hlps[:, 0:256])
                    nc.sync.dma_start(cc_in.ap()[sl, :], hlsb[:])
                    hrps = misc_ps(256)
                    for k in range(2):
                        nc.tensor.matmul(hrps[:, 0:256], lhsT=xnT[:, k * 128:(k + 1) * 128],
                                         rhs=W(f"WrTr{nl}")[:, k * 256:(k + 1) * 256],
                                         start=(k == 0), stop=(k == 1))
                    hrsb = npool.tile([128, H], dt_tab, tag="tabsb", name="tabsb")
                    nc.vector.tensor_copy(hrsb[:], hrps[:, 0:256])
                    nc.sync.dma_start(hr_row.ap()[sl, :], hrsb[:])
                else:
                    xrps = misc_ps(256)
                    for k in range(2):
                        nc.tensor.matmul(xrps[:, 0:256], lhsT=xnT[:, k * 128:(k + 1) * 128],
                                         rhs=W("id0") if k == 0 else W("id1"),
                                         start=(k == 0), stop=(k == 1))
                    xrsb = npool.tile([128, H], dt_tab, tag="tabsb", name="tabsb")
                    nc.vector.tensor_copy(xrsb[:], xrps[:, 0:256])
                    nc.sync.dma_start(x_row.ap()[sl, :], xrsb[:])
                    hmps = misc_ps(256)
                    for k in range(2):
                        nc.tensor.matmul(hmps[:, 0:256], lhsT=xnT[:, k * 128:(k + 1) * 128],
                                         rhs=W("WlTr3")[:, k * 256:(k + 1) * 256],
                                         start=(k == 0), stop=(k == 1))
                    hmsb = npool.tile([128, H], dt_tab, tag="tabsb", name="tabsb")
                    nc.vector.tensor_copy(hmsb[:], hmps[:, 0:256])
                    nc.sync.dma_start(hl_md.ap()[sl, :], hmsb[:])

        edge_layer('gate', 0, xT_a, xT_b)
        tap('x1', xT_b)
        if stop_after != 'x1':
            allgather()
            edge_layer('atom', 0, xT_b, xT_a)
            tap('x2', xT_a)
        if stop_after not in ('x1', 'x2'):
            allgather()
            edge_layer('atom', 1, xT_a, xT_b)
            allgather()
            edge_layer('atom', 2, xT_b, xT_a, last=True)
            tap('x4', xT_a)

        if stop_after in ('x1', 'x2'):
            ob = spool.tile([1, 256], F32, tag="ob")
            nc.vector.memset(ob[:], 0.0)
            nc.sync.dma_start(y.ap()[:, :], ob[:])
            return nc
        # ================= mol phase =================
        glc_cache = []
        for w in range(NWIN):
            t = wpool.tile([128, 1], F32, tag=f"glcc_{w}", name="glcc")
            nc.sync.dma_start(t[:], glocblk.ap()[w])
            glc_cache.append(t)

        ro_ps = pp_acc.tile([128, 1536], F32, tag="aggps", name="rops")
        for w in range(NWIN):
            xr = cpool.tile([128, H], dt_tab, tag="xr")
            nc.sync.dma_start(xr[:], x_row.ap()[w * 128:(w + 1) * 128, :])
            selg = npool.tile([128, NG], F32, tag="selg")
            nc.vector.tensor_scalar(out=selg[:], in0=iota256[:, 0:NG],
                                    scalar1=glc_cache[w][:], scalar2=None,
                                    op0=alu.is_equal)
            for b in range(2):
                nc.tensor.matmul(ro_ps[:, b * 512:b * 512 + NG],
                                 lhsT=xr[:, b * 128:(b + 1) * 128], rhs=selg[:],
                                 start=(w == 0), stop=(w == NWIN - 1))
        outT = wpool.tile([128, 2 * NG], F32, tag="outT0")
        for b in range(2):
            nc.scalar.activation(outT[:, b * NG:(b + 1) * NG],
                                 ro_ps[:, b * 512:b * 512 + NG], act.Relu)

        for step in range(3):
            # hr_m rows [NG, 256] and hrmT [128, 2*NG]
            hrm = molpool.tile([128, NGB * 256], F32, tag="hrm")
            for gb in range(NGB):
                hrps = misc_ps(256)
                for k in range(2):
                    nc.tensor.matmul(
                        hrps[:, 0:256],
                        lhsT=outT[:, k * NG + gb * 128: k * NG + gb * 128 + 128],
                        rhs=W("WrTr3")[:, k * 256:(k + 1) * 256],
                        start=(k == 0), stop=(k == 1))
                nc.vector.tensor_copy(hrm[:, gb * 256:(gb + 1) * 256], hrps[:, 0:256])
            hrmT = molpool.tile([128, 2 * NG], F32, tag="hrmT")
            for b in range(2):
                hrmTps = pp_gru.tile([128, NG], F32, tag="gp", name="hrmTps")
                for k in range(2):
                    nc.tensor.matmul(hrmTps[:],
                                     lhsT=W(f"WrTl3_{k}{b}"),
                                     rhs=outT[:, k * NG:(k + 1) * NG],
                                     start=(k == 0), stop=(k == 1))
                nc.vector.tensor_copy(hrmT[:, b * NG:(b + 1) * NG], hrmTps[:])

            agm_t = pp_acc.tile([128, 1536], F32, tag="aggps", name="agmt")
            sg_ps = agm_t[0:1, 1024:1024 + NG]
            for w in range(NWIN):
                selg = npool.tile([128, NG], F32, tag="selg")
                nc.vector.tensor_scalar(out=selg[:], in0=iota256[:, 0:NG],
                                        scalar1=glc_cache[w][:], scalar2=None,
                                        op0=alu.is_equal)
                Bps = pp_chunk.tile([128, 512], F32, tag="chps")
                for gb in range(NGB):
                    tps = misc_ps(128)
                    nc.tensor.transpose(out=tps[:, 0:128],
                                        in_=selg[:, gb * 128:(gb + 1) * 128],
                                        identity=ident)
                    sTg = spool.tile([128, 128], F32, tag="sTg")
                    nc.vector.tensor_copy(sTg[:], tps[:, 0:128])
                    nc.tensor.matmul(Bps[:, 0:256], lhsT=sTg[:],
                                     rhs=hrm[:, gb * 256:(gb + 1) * 256],
                                     start=(gb == 0), stop=(gb == NGB - 1))
                hmw = cpool.tile([128, H], dt_tab, tag="hmw")
                nc.sync.dma_start(hmw[:], hl_md.ap()[w * 128:(w + 1) * 128, :])
                tt = cpool.tile([128, 256], F32, tag="ttm")
                nc.vector.scalar_tensor_tensor(out=tt[:], in0=Bps[:, 0:256], scalar=0.0,
                                               in1=hmw[:], op0=alu.add, op1=alu.add)
                tlm = cpool.tile([128, 256], F32, tag="tlm")
                nc.scalar.activation(tlm[:], tt[:], act.Lrelu, alpha=0.01)
                escr = cpool.tile([128, 256], F32, tag="escr")
                ecol = spool.tile([128, 1], F32, tag="ecol")
                nc.vector.tensor_tensor(out=escr[:], in0=tlm[:], in1=W("attm_sq"), op=alu.mult)
                nc.vector.reduce_sum(out=ecol[:], in_=escr[:], axis=mybir.AxisListType.X)
                exm = spool.tile([128, 1], F32, tag="ex")
                nc.scalar.activation(exm[:], ecol[:], act.Exp)
                selwm = npool.tile([128, NG], F32, tag="selwm")
                nc.vector.tensor_tensor(out=selwm[:], in0=selg[:],
                                        in1=exm[:].to_broadcast([128, NG]), op=alu.mult)
                for b in range(2):
                    nc.tensor.matmul(agm_t[:, b * 512:b * 512 + NG],
                                     lhsT=tt[:, b * 128:(b + 1) * 128], rhs=selwm[:],
                                     start=(w == 0), stop=(w == NWIN - 1))
                nc.tensor.matmul(sg_ps, lhsT=ones_col, rhs=selwm[:],
                                 start=(w == 0), stop=(w == NWIN - 1))

            smraw = spool.tile([1, NG], F32, tag="smraw")
            nc.vector.tensor_copy(smraw[:], sg_ps)
            srowm = spool.tile([1, NG], F32, tag="srowm")
            nc.vector.tensor_scalar(out=srowm[:], in0=smraw[:], scalar1=EPS,
                                    scalar2=None, op0=alu.max)
            rrowm = spool.tile([1, NG], F32, tag="rrowm")
            nc.vector.reciprocal(rrowm[:], srowm[:])
            rsps = misc_ps(2 * NG)
            nc.tensor.matmul(rsps[:, 0:NG], lhsT=ones1[:], rhs=rrowm[:],
                             start=True, stop=True)
            nc.tensor.matmul(rsps[:, NG:2 * NG], lhsT=ones1[:], rhs=smraw[:],
                             start=True, stop=True)
            rbcm = molpool.tile([128, 2 * NG], F32, tag="rbcm")
            nc.vector.tensor_copy(rbcm[:], rsps[:, 0:2 * NG])
            hTm = molpool.tile([128, 2 * NG], F32, tag="hTm")
            for b in range(2):
                bs = slice(b * NG, (b + 1) * NG)
                t1 = molpool.tile([128, NG], F32, tag="ep_t1m")
                nc.vector.scalar_tensor_tensor(out=t1[:], in0=hrmT[:, bs], scalar=-1.0,
                                               in1=rbcm[:, NG:2 * NG],
                                               op0=alu.mult, op1=alu.mult)
                nc.vector.tensor_tensor(out=t1[:], in0=t1[:],
                                        in1=agm_t[:, b * 512:b * 512 + NG], op=alu.add)
                hpre = molpool.tile([128, NG], F32, tag="ep_hprem")
                nc.vector.tensor_tensor(out=hpre[:], in0=t1[:], in1=rbcm[:, 0:NG],
                                        op=alu.mult)
                nc.vector.tensor_scalar(out=hpre[:], in0=hpre[:],
                                        scalar1=W("molb")[:, b:b + 1],
                                        scalar2=None, op0=alu.add)
                elu(hTm[:, bs], hpre, molpool, NG)
            outT = gru(4, hTm, outT, 2 * NG, NG)

        # ================= MLP =================
        o1ps = misc_ps(NG)
        for k in range(2):
            nc.tensor.matmul(o1ps[:, 0:NG], lhsT=W("w1T")[:, k * 128:(k + 1) * 128],
                             rhs=outT[:, k * NG:(k + 1) * NG],
                             start=(k == 0), stop=(k == 1))
        o1 = npool.tile([128, NG], F32, tag="tabsb", name="o1t")
        nc.scalar.activation(o1[:], o1ps[:, 0:NG], act.Relu, bias=W("b1"))
        o2ps = pp_chunk.tile([64, NG], F32, tag="chps")
        nc.tensor.matmul(o2ps[:], lhsT=W("w2T"), rhs=o1[:], start=True, stop=True)
        o2 = npool.tile([65, NG], F32, tag="tabsb", name="o2t")
        nc.vector.memset(o2[64:65, :], 1.0)
        nc.scalar.activation(o2[0:64, :], o2ps[:], act.Relu, bias=W("b2_")[0:64, :])
        o3ps = pp_gru.tile([1, NG], F32, tag="gp", name="o3ps")
        nc.tensor.matmul(o3ps[:], lhsT=w3T[:], rhs=o2[:], start=True, stop=True)
        o3 = spool.tile([1, NG], F32, tag="o3")
        nc.vector.tensor_copy(o3[:], o3ps[:])
        nc.sync.dma_start(y.ap()[:, 0:NG], o3[:])

    return nc


def make_core_inputs(P, inputs, ci, dt_np=np.float32):
    """Host: per-core input arrays for core ci."""
    c = P['cores'][ci]
    NLOC = P['NLOC']
    x = np.asarray(inputs['x'], np.float32)
    xinT = np.zeros((65, NLOC), np.float32)
    xinT[:64, :c['nn']] = x[c['ns']:c['ne']].T
    xinT[64, :] = 1.0
    ea = np.asarray(inputs['edge_attr'], np.float32)
    ea_perm = np.where(c['sl_fill'][:, None], ea[c['sl_edge']], 0.0).astype(np.float32)
    NCH = P['NCH']
    eaT = ea_perm.reshape(NCH, 128, 16).transpose(0, 2, 1).copy()
    return dict(
        xinT=xinT,
        srcblk=c['srcblk'], dstlocblk=c['dstlocblk'], dstrowblk=c['dstrowblk'],
        glocblk=c['glocblk'], eaT=eaT,
        wpack=make_wpack(inputs),
        lin1T=np.concatenate([np.asarray(inputs['lin1_w'], np.float32).T,
                              np.asarray(inputs['lin1_b'], np.float32)[None, :]], 0),
        W2T=np.asarray(inputs['g_lin1_w'], np.float32)[:, 256:].T.copy(),
        w3T=np.concatenate([np.asarray(inputs['mlp_w3'], np.float32).T,
                            np.asarray(inputs['mlp_b3'], np.float32).reshape(1, 1)], 0),
    )


_CACHE = {}
LAST_EXEC_NS = None

def kernel(**inputs):
    inputs = dict(inputs)
    edge_index = np.asarray(inputs['edge_index']).astype(np.int64)
    batch = np.asarray(inputs['batch']).astype(np.int64)
    n_cores = 8
    G = 2048
    P = preprocess(edge_index, batch, n_cores=n_cores, G=G, CW=5)
    key = (P['NLOC'], P['NWIN'], P['CW'], P['GPC'])
    if key not in _CACHE:
        _CACHE[key] = build_kernel(P['NLOC'], P['NWIN'], P['CW'], P['GPC'], n_cores)
    nc = _CACHE[key]
    ins = [make_core_inputs(P, inputs, ci) for ci in range(n_cores)]
    from concourse.bass_utils import run_bass_kernel_spmd
    trace = bool(os.environ.get('BASS_KERNEL_TRACE'))
    res = run_bass_kernel_spmd(nc, ins, list(range(n_cores)), trace=trace)
    if trace:
        global LAST_EXEC_NS
        LAST_EXEC_NS = res.exec_time_ns
    y = np.concatenate([res.results[c]['y'][0, :P['GPC']] for c in range(n_cores)])
    return y.reshape(G, 1).astype(np.float32)



# revision 6
# speedup vs baseline: 2.0099x; 2.0099x over previous
"""AttentiveFP forward pass as a Bass/Tile kernel on 8 Trainium2 NeuronCores.

v2: bf16 matmuls (4x PE rate + FWL weight loads), host-precomputed edge
selection matrices, aggregation of the gathered hl rows directly (no hrT
reconstruction), GRU batched over 512-node column groups with stationary
weights, Exp-only ACT function in the chunk sweep (Lrelu/Relu/Elu built from
DVE min/max + exp), column-wise reciprocal for the softmax normalizer, and
double-buffered node tables with quarter-chunked AllGathers overlapping
compute.

Data-parallel by graph blocks (256 graphs/core); edges assigned to the core
owning their dst node; per-core windowed segment-softmax aggregation via
selection-matrix matmuls on the PE; per-edge source rows fetched with
indirect DMA gathers from the AllGathered table.
"""
import sys, os
sys.path.insert(0, '/opt/trn_rl_repo')
import numpy as np
import ml_dtypes
from contextlib import ExitStack

import concourse.bass as bass
import concourse.mybir as mybir
import concourse.tile as tile
from concourse.bass import IndirectOffsetOnAxis
from concourse.mybir import AluOpType as alu, ActivationFunctionType as act

BF16 = ml_dtypes.bfloat16
BF = mybir.dt.bfloat16
F32 = mybir.dt.float32
I32 = mybir.dt.int32
EPS = 1e-30

# ---------------- walrus sync-wait splitting ----------------
MAX_WAITS = 1

def split_waits(nc):
    eng_map = nc.engines
    for bbname, bassbb in nc.bb_map.items():
        insts = bassbb.bb.instructions
        i = 0
        while i < len(insts):
            inst = insts[i]
            si = inst.sync_info
            if si is not None and si.on_wait is not None and len(si.on_wait) > MAX_WAITS:
                waits = list(si.on_wait)
                si.on_wait = waits[-MAX_WAITS:]
                rest = waits[:-MAX_WAITS]
                for j in range(0, len(rest), MAX_WAITS):
                    eng = eng_map[inst.engine]
                    nop = eng.nop(nofuse=True)
                    nop_inst = nop.ins
                    for obb in nc.bb_map.values():
                        lst = obb.bb.instructions
                        for k in range(len(lst) - 1, -1, -1):
                            if lst[k].name == nop_inst.name:
                                del lst[k]
                                break
                    nsi = nop_inst.sync_info
                    chunk = rest[j:j + MAX_WAITS]
                    if nsi is None:
                        nop_inst.sync_info = mybir.SyncInfo(on_wait=chunk, on_update=[])
                    else:
                        nsi.on_wait = chunk
                    insts.insert(i, nop_inst)
                    i += 1
            i += 1


class TileContextFixed(tile.TileContext):
    def __exit__(self, *args):
        r = super().__exit__(*args)
        split_waits(self.nc)
        return r


def preprocess(edge_index, batch, n_cores=8, G=2048, CW=5):
    src = np.asarray(edge_index[0]).astype(np.int64)
    dst = np.asarray(edge_index[1]).astype(np.int64)
    batch = np.asarray(batch).astype(np.int64)
    N = batch.shape[0]
    GPC = G // n_cores
    gstart = np.searchsorted(batch, np.arange(0, G + 1, GPC))
    ncounts = np.diff(gstart)
    NLOC = int(np.ceil(ncounts.max() / 512) * 512)
    NWIN = NLOC // 128
    NLQ = NLOC // 4

    node_owner = np.searchsorted(gstart, np.arange(N), side='right') - 1
    loc = np.arange(N) - gstart[node_owner]
    q = loc // NLQ
    ag_row = (q * (n_cores * NLQ) + node_owner * NLQ + (loc % NLQ)).astype(np.int64)
    owner = node_owner[dst]
    ar128 = np.arange(128)

    cores = []
    for c in range(n_cores):
        ns, ne = int(gstart[c]), int(gstart[c + 1])
        nn = ne - ns
        m = owner == c
        eidx = np.nonzero(m)[0]
        dl = dst[eidx] - ns
        order = np.argsort(dl, kind='stable')
        eidx = eidx[order]; dl = dl[order]
        win = dl // 128
        counts = np.bincount(win, minlength=NWIN)
        assert counts.max() <= CW * 128, f"window overflow {counts.max()}"
        pos = np.concatenate([[0], np.cumsum(counts)])[:-1]
        within = np.arange(len(dl)) - pos[win]
        slots = (win * CW * 128 + within).astype(np.int64)

        sl_src = np.zeros(NWIN * CW * 128, np.int32)
        sl_dl = np.full(NWIN * CW * 128, -1.0, np.float32)
        sl_edge = np.zeros(NWIN * CW * 128, np.int64)
        sl_fill = np.zeros(NWIN * CW * 128, bool)
        sl_src[slots] = ag_row[src[eidx]]
        sl_dl[slots] = (dl % 128).astype(np.float32)
        sl_edge[slots] = eidx
        sl_fill[slots] = True

        srcblk = sl_src.reshape(NWIN, CW, 128).transpose(0, 2, 1).copy()
        dlblk = sl_dl.reshape(NWIN, CW, 128).transpose(0, 2, 1).copy()
        # selT [w, d, c*128+p] = 1 if dstloc(w,p,c)==d
        selw = (dlblk.transpose(0, 2, 1)[:, :, None, :] ==
                ar128[None, None, :, None])                  # [w, c, d, p]
        selTblk = selw.transpose(0, 2, 1, 3).reshape(
            NWIN, 128, CW * 128).astype(BF16)

        gloc = np.full(NLOC, -1.0, np.float32)
        gloc[:nn] = (batch[ns:ne] - c * GPC).astype(np.float32)
        glocT = gloc.reshape(NWIN, 128).T.copy()             # [128, NWIN]
        gw = gloc.reshape(NWIN, 128)
        selgT = np.zeros((NWIN, 128, 256), BF16)
        for gb in range(2):
            selgT[:, :, gb * 128:(gb + 1) * 128] = (
                gw[:, None, :] == (gb * 128 + ar128)[None, :, None])

        cores.append(dict(ns=ns, ne=ne, nn=nn,
                          srcblk=srcblk, dlblk=dlblk, selTblk=selTblk,
                          glocT=glocT, selgT=selgT,
                          sl_edge=sl_edge, sl_fill=sl_fill))
    return dict(cores=cores, gstart=gstart, NLOC=NLOC, NWIN=NWIN, NLQ=NLQ,
                CW=CW, GPC=GPC, n_cores=n_cores)


def wpb_layout():
    """bf16 pack: name -> (off, cols)."""
    L = {}
    off = 0
    def add(name, cols):
        nonlocal off
        L[name] = (off, cols)
        off += cols
    add("iota_sq", 128)
    add("iota256", 256)
    add("ones_col", 1)
    add("ident", 128)
    add("attg", 256)
    for l in range(3):
        add(f"atta{l}", 256)
    add("attm", 256)
    for b in range(2):
        add(f"W1A{b}", 257)
    for i in range(4):
        add(f"WlTr{i}", 512)
        add(f"WrTr{i}", 512)
    add("id0", 256)
    add("id1", 256)
    for k in range(2):
        for b in range(2):
            add(f"gl2T_{k}{b}", 128)
    for g in range(5):
        for j in range(12):
            for b in range(2):
                add(f"gru{g}_w{j}{b}", 128)
    add("w1T", 256)
    add("w2T", 64)
    return L, off


def wpf_layout():
    """f32 pack (biases): name -> (off, cols)."""
    L = {}
    off = 0
    def add(name, cols):
        nonlocal off
        L[name] = (off, cols)
        off += cols
    for g in range(5):
        for j in range(4):
            for b in range(2):
                add(f"gru{g}_b{j}{b}", 1)
    for l in range(3):
        for b in range(2):
            add(f"ab{l}{b}", 1)
    for b in range(2):
        add(f"gb2{b}", 1)
    for b in range(2):
        add(f"molb{b}", 1)
    add("b1", 1)
    add("b2_", 1)
    return L, off


def make_wpacks(inp):
    Lb, WB = wpb_layout()
    Lf, WF = wpf_layout()
    Wb = np.zeros((128, WB), np.float32)
    Wf = np.zeros((128, WF), np.float32)

    def putb(name, arr):
        off, cols = Lb[name]
        assert arr.shape == (128, cols), (name, arr.shape, cols)
        Wb[:, off:off + cols] = arr

    def putf(name, arr):
        off, cols = Lf[name]
        assert arr.shape == (128, cols), (name, arr.shape, cols)
        Wf[:, off:off + cols] = arr

    putb("iota_sq", np.tile(np.arange(128, dtype=np.float32), (128, 1)))
    putb("iota256", np.tile(np.arange(256, dtype=np.float32), (128, 1)))
    putb("ones_col", np.ones((128, 1), np.float32))
    I = np.eye(128, dtype=np.float32)
    putb("ident", I)
    putb("attg", np.tile(np.asarray(inp['g_att_l'], np.float32), (128, 1)))
    for l in range(3):
        putb(f"atta{l}", np.tile(np.asarray(inp['atom_att'][l], np.float32), (128, 1)))
    putb("attm", np.tile(np.asarray(inp['mol_att'], np.float32), (128, 1)))
    W1T = np.asarray(inp['g_lin1_w'], np.float32)[:, :256].T     # [256 in, 256 out]
    attr = np.asarray(inp['g_att_r'], np.float32).reshape(2, 128).T  # [128, 2]
    for b in range(2):
        putb(f"W1A{b}", np.concatenate(
            [W1T[b * 128:(b + 1) * 128], attr[:, b:b + 1]], axis=1))
    Wls = [np.asarray(inp['atom_Wl'][0]), np.asarray(inp['atom_Wl'][1]),
           np.asarray(inp['atom_Wl'][2]), np.asarray(inp['mol_Wl'])]
    Wrs = [np.asarray(inp['atom_Wr'][0]), np.asarray(inp['atom_Wr'][1]),
           np.asarray(inp['atom_Wr'][2]), np.asarray(inp['mol_Wr'])]
    for i in range(4):
        WT = Wls[i].T.astype(np.float32)                         # [256 k, 256 out]
        putb(f"WlTr{i}", np.concatenate([WT[0:128], WT[128:256]], axis=1))
        WT = Wrs[i].T.astype(np.float32)
        putb(f"WrTr{i}", np.concatenate([WT[0:128], WT[128:256]], axis=1))
    putb("id0", np.concatenate([I, np.zeros((128, 128), np.float32)], 1))
    putb("id1", np.concatenate([np.zeros((128, 128), np.float32), I], 1))
    g2T = np.asarray(inp['g_lin2_w'], np.float32).T              # [k, out]
    for k in range(2):
        for b in range(2):
            putb(f"gl2T_{k}{b}", g2T[k * 128:(k + 1) * 128, b * 128:(b + 1) * 128])
    grus = [('gru0_wih', 'gru0_whh', 'gru0_bih', 'gru0_bhh', None),
            ('agru_wih', 'agru_whh', 'agru_bih', 'agru_bhh', 0),
            ('agru_wih', 'agru_whh', 'agru_bih', 'agru_bhh', 1),
            ('agru_wih', 'agru_whh', 'agru_bih', 'agru_bhh', 2),
            ('mgru_wih', 'mgru_whh', 'mgru_bih', 'mgru_bhh', None)]
    for g, (wi, wh, bi, bh, l) in enumerate(grus):
        wih = np.asarray(inp[wi] if l is None else inp[wi][l], np.float32)
        whh = np.asarray(inp[wh] if l is None else inp[wh][l], np.float32)
        bih = np.asarray(inp[bi] if l is None else inp[bi][l], np.float32)
        bhh = np.asarray(inp[bh] if l is None else inp[bh][l], np.float32)
        wihT = wih.T    # [256 k, 768]
        whhT = whh.T
        # j: 0,1 wih-r (input side); 2,3 whh-r (hidden side); 4..7 z; 8,9 wih-n; 10,11 whh-n
        for k in range(2):
            for b in range(2):
                ks, bs = slice(k * 128, (k + 1) * 128), slice(b * 128, (b + 1) * 128)
                putb(f"gru{g}_w{0 + k}{b}", wihT[ks, 0:256][:, bs])
                putb(f"gru{g}_w{2 + k}{b}", whhT[ks, 0:256][:, bs])
                putb(f"gru{g}_w{4 + k}{b}", wihT[ks, 256:512][:, bs])
                putb(f"gru{g}_w{6 + k}{b}", whhT[ks, 256:512][:, bs])
                putb(f"gru{g}_w{8 + k}{b}", wihT[ks, 512:768][:, bs])
                putb(f"gru{g}_w{10 + k}{b}", whhT[ks, 512:768][:, bs])
        br = (bih[0:256] + bhh[0:256]).reshape(2, 128).T
        bz = (bih[256:512] + bhh[256:512]).reshape(2, 128).T
        bin_ = bih[512:768].reshape(2, 128).T
        bhn = bhh[512:768].reshape(2, 128).T
        for j, arr in enumerate([br, bz, bin_, bhn]):
            for b in range(2):
                putf(f"gru{g}_b{j}{b}", arr[:, b:b + 1])
    ab = np.asarray(inp['atom_bias'], np.float32)
    for l in range(3):
        for b in range(2):
            putf(f"ab{l}{b}", ab[l].reshape(2, 128).T[:, b:b + 1])
    gb2 = np.asarray(inp['g_bias'], np.float32).reshape(2, 128).T
    molb = np.asarray(inp['mol_bias'], np.float32).reshape(2, 128).T
    for b in range(2):
        putf(f"gb2{b}", gb2[:, b:b + 1])
        putf(f"molb{b}", molb[:, b:b + 1])
    putf("b1", np.asarray(inp['mlp_b1'], np.float32).reshape(128, 1))
    putf("b2_", np.pad(np.asarray(inp['mlp_b2'], np.float32), (0, 64)).reshape(128, 1))
    w1T = np.asarray(inp['mlp_w1'], np.float32).T                # [256, 128]
    putb("w1T", np.concatenate([w1T[0:128], w1T[128:256]], 1))
    putb("w2T", np.asarray(inp['mlp_w2'], np.float32).T)         # [128, 64]
    return Wb.astype(BF16), Wf


def build_kernel(NLOC, NWIN, CW, NG, n_cores):
    H = 256
    NLQ = NLOC // 4
    NGRP = NWIN // 4
    NWQ = NWIN // 4            # windows per AG quarter

    nc = bass.Bass(num_devices=n_cores)
    Lb, WB = wpb_layout()
    Lf, WF = wpf_layout()

    def dram_in(name, shape, dt=BF):
        return nc.dram_tensor(name, list(shape), dt, kind="ExternalInput")

    xinT = dram_in("xinT", [65, NLOC])
    srcblk = dram_in("srcblk", [NWIN, 128, CW], I32)
    dlblk = dram_in("dlblk", [NWIN, 128, CW], F32)
    selTblk = dram_in("selTblk", [NWIN, 128, CW * 128])
    eaT2 = dram_in("eaT2", [NWIN, 16, CW * 128])
    glocTd = dram_in("glocT", [128, NWIN], F32)
    selgTblk = dram_in("selgTblk", [NWIN, 128, 256])
    wpbd = dram_in("wpb", [128, WB])
    wpfd = dram_in("wpf", [128, WF], F32)
    lin1Td = dram_in("lin1T", [65, 256])
    W2Td = dram_in("W2T", [16, 256])
    w3Td = dram_in("w3T", [65, 1])

    y = nc.dram_tensor("y", [1, 256], F32, kind="ExternalOutput")

    cc_in = nc.dram_tensor("cc_in", [NLOC, H], BF)
    tabs = [nc.dram_tensor(f"tab{i}", [n_cores * NLOC, H], BF, addr_space="Shared")
            for i in range(2)]
    xld = nc.dram_tensor("xld", [NLOC, H], BF)

    with TileContextFixed(nc) as tc, ExitStack() as ctx:
        wpool = ctx.enter_context(tc.tile_pool(name="weights", bufs=1))
        mpool = ctx.enter_context(tc.tile_pool(name="meta", bufs=2))
        gpool = ctx.enter_context(tc.tile_pool(name="gath", bufs=6))
        vpool = ctx.enter_context(tc.tile_pool(name="vals", bufs=6))
        npool = ctx.enter_context(tc.tile_pool(name="node", bufs=3))
        spool = ctx.enter_context(tc.tile_pool(name="small", bufs=4))
        upool = ctx.enter_context(tc.tile_pool(name="gru", bufs=2))
        pp = ctx.enter_context(tc.tile_pool(name="ps", bufs=1, space="PSUM"))

        wpb = wpool.tile([128, WB], BF, tag="wpb")
        nc.sync.dma_start(wpb[:], wpbd.ap())
        wpf = wpool.tile([128, WF], F32, tag="wpf")
        nc.sync.dma_start(wpf[:], wpfd.ap())

        def W(name):
            off, cols = Lb[name]
            return wpb[:, off:off + cols]

        def F(name):
            off, cols = Lf[name]
            return wpf[:, off:off + cols]

        iota_sq = W("iota_sq")
        ident = W("ident")
        ones_col = W("ones_col")
        lin1T = wpool.tile([65, 256], BF, tag="lin1T")
        nc.sync.dma_start(lin1T[:], lin1Td.ap())
        W2T = wpool.tile([16, 256], BF, tag="W2T")
        nc.sync.dma_start(W2T[:], W2Td.ap())
        w3T = wpool.tile([65, 1], BF, tag="w3T")
        nc.sync.dma_start(w3T[:], w3Td.ap())
        ones1 = wpool.tile([1, 128], BF, tag="ones1")
        nc.vector.memset(ones1[:], 1.0)
        onesq = wpool.tile([1, 1], BF, tag="onesq")
        nc.vector.memset(onesq[:], 1.0)
        glocT = wpool.tile([128, NWIN], F32, tag="glocT")
        nc.sync.dma_start(glocT[:], glocTd.ap())

        # persistent node-state tiles
        xts = [wpool.tile([128, NLOC], BF, tag=f"xT{b}", name=f"xT{b}") for b in range(2)]
        hrR = wpool.tile([128, NWIN * 256], BF, tag="hrR")
        wc = wpool.tile([128, NWIN], BF, tag="wc")
        outTs = [wpool.tile([128, NG], BF, tag=f"outT{b}", name=f"outT{b}") for b in range(2)]

        def ps_tile(tag, name):
            return pp.tile([128, 512], F32, tag=tag, name=name, bufs={
                "agg": 2, "ch": 2, "gp": 3}[tag])

        def launch_ag(dst_tab, q):
            rs = cc_in.ap()[q * NLQ:(q + 1) * NLQ, :]
            os_ = dst_tab.ap()[q * n_cores * NLQ:(q + 1) * n_cores * NLQ, :]
            nc.gpsimd.collective_compute(
                "AllGather", alu.bypass,
                replica_groups=[list(range(n_cores))],
                ins=[rs], outs=[os_])

        def ag_quarters(grp):
            return [q for q in range(4) if ((q + 1) * NWQ - 1) // 4 == grp]

        # ---------- softmax normalizer: row sums -> rbc [128,128] ----------
        def make_rbc(agg, sums_cols):
            """agg bank holds row [1,128] of exp-sums at sums_cols; returns
            rbc sbuf tile [128,128] bf16 with 1/sum broadcast down columns."""
            sums_sb = spool.tile([1, 128], BF, tag="sums_sb", name="sums_sb")
            nc.vector.tensor_scalar(out=sums_sb[:], in0=agg[0:1, sums_cols],
                                    scalar1=EPS, scalar2=None, op0=alu.max)
            cps = ps_tile("ch", "colps")
            nc.tensor.matmul(cps[:, 0:1], lhsT=sums_sb[:], rhs=onesq[:],
                             start=True, stop=True)
            recipc = spool.tile([128, 1], BF, tag="recipc", name="recipc")
            with nc.allow_low_precision(reason="bf16 softmax normalizer"):
                nc.vector.reciprocal(recipc[:], cps[:, 0:1])
            rps = ps_tile("ch", "rowps")
            nc.tensor.matmul(rps[0:1, 0:128], lhsT=recipc[:], rhs=ident,
                             start=True, stop=True)
            rrow = spool.tile([1, 128], BF, tag="rrow", name="rrow")
            nc.vector.tensor_copy(rrow[:], rps[0:1, 0:128])
            bps = ps_tile("ch", "bcps")
            nc.tensor.matmul(bps[:, 0:128], lhsT=ones1[:], rhs=rrow[:],
                             start=True, stop=True)
            rbc = npool.tile([128, 128], BF, tag="rbc", name="rbc")
            nc.vector.tensor_copy(rbc[:], bps[:, 0:128])
            return rbc

        def elu_into(dst_ap, t, bias_ap, fd):
            """dst = elu(t + bias); t is sbuf bf16 [128, fd]."""
            m = npool.tile([128, fd], BF, tag=f"elu_m{fd}", name="elum")
            nc.vector.tensor_scalar(out=m[:], in0=t[:], scalar1=bias_ap,
                                    scalar2=0.0, op0=alu.add, op1=alu.min)
            r = npool.tile([128, fd], BF, tag=f"elu_r{fd}", name="elur")
            nc.vector.tensor_scalar(out=r[:], in0=t[:], scalar1=bias_ap,
                                    scalar2=0.0, op0=alu.add, op1=alu.max)
            e = npool.tile([128, fd], BF, tag=f"elu_e{fd}", name="elue")
            nc.scalar.activation(e[:], m[:], act.Exp)
            nc.vector.scalar_tensor_tensor(out=dst_ap, in0=e[:], scalar=-1.0,
                                           in1=r[:], op0=alu.add, op1=alu.add)

        # ---------------- batched GRU ----------------
        def gru_batched(g, hTg, kspan, xsl, ncols):
            """hTg: [128, 2*kspan] bf16 (input feature half k at cols k*kspan);
            hidden state = xts-like tiles given by closure target `gxts`;
            writes relu'd new state back into gxts[b][:, xsl]."""
            gxts = xts if g < 4 else outTs

            def gate_ps(b, jh, jx, name):
                p = ps_tile("gp", name)
                nmm = (2 if jh is not None else 0) + (2 if jx is not None else 0)
                i = 0
                for k in range(2):
                    if jh is not None:
                        nc.tensor.matmul(p[:, 0:ncols],
                                         lhsT=W(f"gru{g}_w{jh + k}{b}"),
                                         rhs=hTg[:, k * kspan:k * kspan + ncols],
                                         start=(i == 0), stop=(i == nmm - 1))
                        i += 1
                for k in range(2):
                    if jx is not None:
                        nc.tensor.matmul(p[:, 0:ncols],
                                         lhsT=W(f"gru{g}_w{jx + k}{b}"),
                                         rhs=gxts[k][:, xsl],
                                         start=(i == 0), stop=(i == nmm - 1))
                        i += 1
                return p

            rps = [gate_ps(b, 0, 2, "rps") for b in range(2)]
            r = []
            for b in range(2):
                t = upool.tile([128, ncols], BF, tag="gru_r", name="grur")
                nc.scalar.activation(t[:], rps[b][:, 0:ncols], act.Sigmoid,
                                     bias=F(f"gru{g}_b0{b}"))
                r.append(t)
            zps = [gate_ps(b, 4, 6, "zps") for b in range(2)]
            z = []
            for b in range(2):
                t = upool.tile([128, ncols], BF, tag="gru_z", name="gruz")
                nc.scalar.activation(t[:], zps[b][:, 0:ncols], act.Sigmoid,
                                     bias=F(f"gru{g}_b1{b}"))
                z.append(t)
            hps = [gate_ps(b, None, 10, "hps") for b in range(2)]
            t1 = []
            for b in range(2):
                t = upool.tile([128, ncols], BF, tag="gru_t1", name="grut1")
                nc.vector.scalar_tensor_tensor(out=t[:], in0=hps[b][:, 0:ncols],
                                               scalar=F(f"gru{g}_b3{b}"),
                                               in1=r[b][:], op0=alu.add,
                                               op1=alu.mult)
                t1.append(t)
            ips = [gate_ps(b, 8, None, "ips") for b in range(2)]
            t2 = []
            for b in range(2):
                t = upool.tile([128, ncols], BF, tag="gru_t2", name="grut2")
                nc.vector.tensor_tensor(out=t[:], in0=ips[b][:, 0:ncols],
                                        in1=t1[b][:], op=alu.add)
                t2.append(t)
            n = []
            for b in range(2):
                t = upool.tile([128, ncols], BF, tag="gru_n", name="grun")
                nc.scalar.activation(t[:], t2[b][:], act.Tanh,
                                     bias=F(f"gru{g}_b2{b}"))
                n.append(t)
            for b in range(2):
                d = upool.tile([128, ncols], BF, tag="gru_d", name="grud")
                nc.vector.tensor_tensor(out=d[:], in0=gxts[b][:, xsl],
                                        in1=n[b][:], op=alu.subtract)
                zd = upool.tile([128, ncols], BF, tag="gru_zd", name="gruzd")
                nc.vector.tensor_tensor(out=zd[:], in0=z[b][:], in1=d[:],
                                        op=alu.mult)
                s = upool.tile([128, ncols], BF, tag="gru_s", name="grus")
                nc.vector.tensor_tensor(out=s[:], in0=n[b][:], in1=zd[:],
                                        op=alu.add)
                nc.vector.tensor_scalar(out=gxts[b][:, xsl], in0=s[:],
                                        scalar1=0.0, scalar2=None, op0=alu.max)

        # ---------------- table production (per window) ----------------
        def table_rows(li, w, last):
            wsl = slice(w * 128, (w + 1) * 128)
            p = ps_tile("ch", "tabps")
            for k in range(2):
                nc.tensor.matmul(p[:, 0:256], lhsT=xts[k][:, wsl],
                                 rhs=W(f"WlTr{li}")[:, k * 256:(k + 1) * 256],
                                 start=(k == 0), stop=(k == 1))
            if last:
                # hl_m rows for the mol phase live in hrR
                nc.vector.tensor_copy(hrR[:, w * 256:(w + 1) * 256], p[:, 0:256])
                p2 = ps_tile("ch", "tabps2")
                for k in range(2):
                    nc.tensor.matmul(p2[:, 0:256], lhsT=xts[k][:, wsl],
                                     rhs=W("id0") if k == 0 else W("id1"),
                                     start=(k == 0), stop=(k == 1))
                xsb = npool.tile([128, 256], BF, tag="cp", name="xsb")
                nc.vector.tensor_copy(xsb[:], p2[:, 0:256])
                nc.sync.dma_start(xld.ap()[wsl, :], xsb[:])
            else:
                hsb = npool.tile([128, 256], BF, tag="cp", name="hsb")
                nc.vector.tensor_copy(hsb[:], p[:, 0:256])
                nc.sync.dma_start(cc_in.ap()[wsl, :], hsb[:])
                p2 = ps_tile("ch", "tabps2")
                for k in range(2):
                    nc.tensor.matmul(p2[:, 0:256], lhsT=xts[k][:, wsl],
                                     rhs=W(f"WrTr{li}")[:, k * 256:(k + 1) * 256],
                                     start=(k == 0), stop=(k == 1))
                nc.vector.tensor_copy(hrR[:, w * 256:(w + 1) * 256], p2[:, 0:256])

        # ================= P0: input projection + u table =================
        for grp in range(NGRP):
            gsl = slice(grp * 512, (grp + 1) * 512)
            xing = mpool.tile([65, 512], BF, tag="xing")
            nc.sync.dma_start(xing[:], xinT.ap()[:, gsl])
            for b in range(2):
                p = ps_tile("gp", "p0ps")
                nc.tensor.matmul(p[:, 0:512], lhsT=lin1T[:, b * 128:(b + 1) * 128],
                                 rhs=xing[:], start=True, stop=True)
                nc.scalar.activation(xts[b][:, gsl], p[:, 0:512], act.Lrelu,
                                     alpha=0.01)
            for wi in range(4):
                w = grp * 4 + wi
                wsl = slice(w * 128, (w + 1) * 128)
                p = ps_tile("ch", "ups")
                for b in range(2):
                    nc.tensor.matmul(p[:, 0:257], lhsT=xts[b][:, wsl],
                                     rhs=W(f"W1A{b}"), start=(b == 0),
                                     stop=(b == 1))
                usb = npool.tile([128, 256], BF, tag="cp", name="usb")
                nc.vector.tensor_copy(usb[:], p[:, 0:256])
                nc.sync.dma_start(cc_in.ap()[wsl, :], usb[:])
                nc.vector.tensor_copy(wc[:, w:w + 1], p[:, 256:257])
            for q in ag_quarters(grp):
                launch_ag(tabs[0], q)

        # ================= edge layers =================
        def edge_layer(li, kind, src_tab, has_ag):
            gate = kind == 'gate'
            attw = W("attg") if gate else W(f"atta{li - 1}")
            dst_tab = tabs[(li + 1) % 2]
            for grp in range(NGRP):
                hTg = upool.tile([128, 1024], BF, tag="hTg", name="hTg")
                for wi in range(4):
                    w = grp * 4 + wi
                    wsl = slice(w * 128, (w + 1) * 128)
                    srct = mpool.tile([128, CW], I32, tag="srct")
                    nc.sync.dma_start(srct[:], srcblk.ap()[w])
                    dlc = mpool.tile([128, CW], F32, tag="dlc")
                    nc.sync.dma_start(dlc[:], dlblk.ap()[w])
                    selTw = mpool.tile([128, CW * 128], BF, tag="selTw")
                    nc.sync.dma_start(selTw[:], selTblk.ap()[w])
                    if gate:
                        eatw = mpool.tile([16, CW * 128], BF, tag="eatw")
                        nc.sync.dma_start(eatw[:], eaT2.ap()[w])

                    agg = ps_tile("agg", "agg")
                    ec = spool.tile([128, CW], F32, tag="ec", name="ec")
                    vals = []
                    for ci in range(CW):
                        csl = slice(ci * 128, (ci + 1) * 128)
                        g = gpool.tile([128, H], BF, tag="g", name="g")
                        nc.gpsimd.indirect_dma_start(
                            out=g[:], out_offset=None, in_=src_tab.ap(),
                            in_offset=IndirectOffsetOnAxis(
                                ap=srct[:, ci:ci + 1], axis=0))
                        ch = ps_tile("ch", "chps")
                        if gate:
                            nc.tensor.matmul(ch[:, 0:256], lhsT=eatw[:, csl],
                                             rhs=W2T[:], start=True, stop=False)
                        else:
                            nc.tensor.matmul(ch[:, 0:256], lhsT=selTw[:, csl],
                                             rhs=hrR[:, w * 256:(w + 1) * 256],
                                             start=True, stop=False)
                        nc.tensor.matmul(ch[:, 0:256], lhsT=ident, rhs=g[:],
                                         start=False, stop=True)
                        if gate:
                            # (x @ att_r)[dst] column; first touch of agg bank
                            nc.tensor.matmul(agg[:, 384 + ci:385 + ci],
                                             lhsT=selTw[:, csl],
                                             rhs=wc[:, w:w + 1],
                                             start=(ci == 0), stop=False,
                                             skip_group_check=True)
                        tsb = npool.tile([128, H], BF, tag="tsb", name="tsb")
                        nc.vector.tensor_copy(tsb[:], ch[:, 0:256])
                        if gate:
                            tl = vpool.tile([128, H], BF, tag="val", name="tl")
                        else:
                            tl = npool.tile([128, H], BF, tag="tls", name="tl")
                        nc.vector.scalar_tensor_tensor(
                            out=tl[:], in0=tsb[:], scalar=0.01, in1=tsb[:],
                            op0=alu.mult, op1=alu.max)
                        escr = npool.tile([128, H], BF, tag="escr", name="escr")
                        nc.vector.scalar_tensor_tensor(
                            out=escr[:], in0=tl[:], scalar=1.0, in1=attw,
                            op0=alu.mult, op1=alu.mult,
                            accum_out=ec[:, ci:ci + 1])
                        vals.append(tl if gate else g)

                    if gate:
                        e2 = spool.tile([128, CW], F32, tag="e2", name="e2")
                        nc.vector.scalar_tensor_tensor(
                            out=e2[:], in0=ec[:], scalar=0.0,
                            in1=agg[:, 384:384 + CW], op0=alu.add, op1=alu.add)
                        el = spool.tile([128, CW], F32, tag="el", name="el")
                        nc.vector.scalar_tensor_tensor(
                            out=el[:], in0=e2[:], scalar=0.01, in1=e2[:],
                            op0=alu.mult, op1=alu.max)
                        ex = spool.tile([128, CW], F32, tag="ex", name="ex")
                        nc.scalar.activation(ex[:], el[:], act.Exp)
                    else:
                        ex = spool.tile([128, CW], F32, tag="ex", name="ex")
                        nc.scalar.activation(ex[:], ec[:], act.Exp)

                    for ci in range(CW):
                        selw = spool.tile([128, 128], BF, tag="selw", name="selw")
                        nc.vector.tensor_scalar(
                            out=selw[:], in0=iota_sq, scalar1=dlc[:, ci:ci + 1],
                            scalar2=ex[:, ci:ci + 1], op0=alu.is_equal,
                            op1=alu.mult)
                        first = (ci == 0) and not gate
                        for b in range(2):
                            nc.tensor.matmul(
                                agg[:, b * 128:(b + 1) * 128],
                                lhsT=vals[ci][:, b * 128:(b + 1) * 128],
                                rhs=selw[:], start=(first and b == 0),
                                stop=(ci == CW - 1), skip_group_check=True)
                        nc.tensor.matmul(agg[0:1, 256:384], lhsT=ones_col,
                                         rhs=selw[:], start=False,
                                         stop=(ci == CW - 1),
                                         skip_group_check=True)

                    # ---- epilogue ----
                    rbc = make_rbc(agg, slice(256, 384))
                    if gate:
                        aggn = []
                        for b in range(2):
                            t = npool.tile([128, 128], BF, tag=f"aggn{b}",
                                           name="aggn")
                            nc.vector.tensor_tensor(
                                out=t[:], in0=agg[:, b * 128:(b + 1) * 128],
                                in1=rbc[:], op=alu.mult)
                            aggn.append(t)
                        h0 = ps_tile("ch", "h0ps")
                        for b in range(2):
                            for k in range(2):
                                nc.tensor.matmul(
                                    h0[:, b * 256:b * 256 + 128],
                                    lhsT=W(f"gl2T_{k}{b}"), rhs=aggn[k][:],
                                    start=(b == 0 and k == 0),
                                    stop=(b == 1 and k == 1),
                                    skip_group_check=True)
                        for b in range(2):
                            t = npool.tile([128, 128], BF, tag="tb", name="tb")
                            nc.vector.tensor_copy(t[:], h0[:, b * 256:b * 256 + 128])
                            elu_into(hTg[:, b * 512 + wi * 128:
                                         b * 512 + wi * 128 + 128],
                                     t, F(f"gb2{b}"), 128)
                    else:
                        for b in range(2):
                            t = npool.tile([128, 128], BF, tag="tb", name="tb")
                            nc.vector.tensor_tensor(
                                out=t[:], in0=agg[:, b * 128:(b + 1) * 128],
                                in1=rbc[:], op=alu.mult)
                            elu_into(hTg[:, b * 512 + wi * 128:
                                         b * 512 + wi * 128 + 128],
                                     t, F(f"ab{li - 1}{b}"), 128)

                # ---- GRU over the 4-window group ----
                gru_batched(li, hTg, 512, slice(grp * 512, (grp + 1) * 512), 512)
                # ---- next-layer tables ----
                for wi in range(4):
                    table_rows(li, grp * 4 + wi, last=(li == 3))
                if has_ag:
                    for q in ag_quarters(grp):
                        launch_ag(dst_tab, q)

        edge_layer(0, 'gate', tabs[0], True)
        edge_layer(1, 'atom', tabs[1], True)
        edge_layer(2, 'atom', tabs[0], True)
        edge_layer(3, 'atom', tabs[1], False)

        # ================= mol phase =================
        # readout: out0 = relu(segment_sum(x))
        ro = [ps_tile("agg", "ro0"), ps_tile("agg", "ro1")]
        for w in range(NWIN):
            wsl = slice(w * 128, (w + 1) * 128)
            xr = mpool.tile([128, H], BF, tag="xr")
            nc.sync.dma_start(xr[:], xld.ap()[wsl, :])
            selg = npool.tile([128, NG], BF, tag="selg", name="selg")
            nc.vector.tensor_scalar(out=selg[:], in0=W("iota256")[:, 0:NG],
                                    scalar1=glocT[:, w:w + 1], scalar2=None,
                                    op0=alu.is_equal)
            for b in range(2):
                nc.tensor.matmul(ro[b][:, 0:NG],
                                 lhsT=xr[:, b * 128:(b + 1) * 128],
                                 rhs=selg[:], start=(w == 0),
                                 stop=(w == NWIN - 1))
        for b in range(2):
            nc.vector.tensor_scalar(out=outTs[b][:], in0=ro[b][:, 0:NG],
                                    scalar1=0.0, scalar2=None, op0=alu.max)

        for step in range(3):
            # hr rows for graphs  [2 x (128 g, 256 f)]
            hrm = []
            for gb in range(2):
                p = ps_tile("ch", "hrmps")
                for k in range(2):
                    nc.tensor.matmul(p[:, 0:256],
                                     lhsT=outTs[k][:, gb * 128:(gb + 1) * 128],
                                     rhs=W("WrTr3")[:, k * 256:(k + 1) * 256],
                                     start=(k == 0), stop=(k == 1))
                t = upool.tile([128, 256], BF, tag=f"hrm{gb}", name="hrm")
                nc.vector.tensor_copy(t[:], p[:, 0:256])
                hrm.append(t)

            agA = ps_tile("agg", "agA")   # agm0 @0:256, sums @256:512
            agB = ps_tile("agg", "agB")   # agm1 @0:256
            nw4 = (NWIN + 3) // 4
            for g4 in range(nw4):
                wlist = range(g4 * 4, min((g4 + 1) * 4, NWIN))
                ecm = spool.tile([128, 4], F32, tag="ecm", name="ecm")
                tls = []
                for i, w in enumerate(wlist):
                    wsl = slice(w * 128, (w + 1) * 128)
                    selgTw = mpool.tile([128, 256], BF, tag="selgTw")
                    nc.sync.dma_start(selgTw[:], selgTblk.ap()[w])
                    ch = ps_tile("ch", "chps")
                    for gb in range(2):
                        nc.tensor.matmul(ch[:, 0:256],
                                         lhsT=selgTw[:, gb * 128:(gb + 1) * 128],
                                         rhs=hrm[gb][:], start=(gb == 0),
                                         stop=False)
                    nc.tensor.matmul(ch[:, 0:256], lhsT=ident,
                                     rhs=hrR[:, w * 256:(w + 1) * 256],
                                     start=False, stop=True)
                    tsb = npool.tile([128, H], BF, tag="tsb", name="tsbm")
                    nc.vector.tensor_copy(tsb[:], ch[:, 0:256])
                    tlm = npool.tile([128, H], BF, tag="tls", name="tlm")
                    nc.vector.scalar_tensor_tensor(
                        out=tlm[:], in0=tsb[:], scalar=0.01, in1=tsb[:],
                        op0=alu.mult, op1=alu.max)
                    escr = npool.tile([128, H], BF, tag="escr", name="escrm")
                    nc.vector.scalar_tensor_tensor(
                        out=escr[:], in0=tlm[:], scalar=1.0, in1=W("attm"),
                        op0=alu.mult, op1=alu.mult,
                        accum_out=ecm[:, i:i + 1])
                exm = spool.tile([128, 4], F32, tag="exm", name="exm")
                nc.scalar.activation(exm[:, 0:len(list(wlist))],
                                     ecm[:, 0:len(list(wlist))], act.Exp)
                for i, w in enumerate(wlist):
                    selwm = npool.tile([128, NG], BF, tag="selwm", name="selwm")
                    nc.vector.tensor_scalar(
                        out=selwm[:], in0=W("iota256")[:, 0:NG],
                        scalar1=glocT[:, w:w + 1], scalar2=exm[:, i:i + 1],
                        op0=alu.is_equal, op1=alu.mult)
                    first = (w == 0)
                    last = (w == NWIN - 1)
                    nc.tensor.matmul(agA[:, 0:NG],
                                     lhsT=hrR[:, w * 256:w * 256 + 128],
                                     rhs=selwm[:], start=first, stop=last,
                                     skip_group_check=True)
                    nc.tensor.matmul(agB[:, 0:NG],
                                     lhsT=hrR[:, w * 256 + 128:(w + 1) * 256],
                                     rhs=selwm[:], start=first, stop=last,
                                     skip_group_check=True)
                    nc.tensor.matmul(agA[0:1, NG:2 * NG], lhsT=ones_col,
                                     rhs=selwm[:], start=False, stop=last,
                                     skip_group_check=True)

            # mol epilogue
            sums_sb = spool.tile([1, NG], BF, tag="sumsm", name="sumsm")
            nc.vector.tensor_scalar(out=sums_sb[:], in0=agA[0:1, NG:2 * NG],
                                    scalar1=EPS, scalar2=None, op0=alu.max)
            rrow = spool.tile([1, NG], BF, tag="rrowm", name="rrowm")
            with nc.allow_low_precision(reason="bf16 softmax normalizer"):
                nc.vector.reciprocal(rrow[:], sums_sb[:])
            bps = ps_tile("ch", "bcpsm")
            nc.tensor.matmul(bps[:, 0:NG], lhsT=ones1[:], rhs=rrow[:],
                             start=True, stop=True)
            rbcm = upool.tile([128, NG], BF, tag="rbcm", name="rbcm")
            nc.vector.tensor_copy(rbcm[:], bps[:, 0:NG])
            hTmg = upool.tile([128, 2 * NG], BF, tag="hTmg", name="hTmg")
            for b in range(2):
                ag_b = agA if b == 0 else agB
                t = upool.tile([128, NG], BF, tag="tbm", name="tbm")
                nc.vector.tensor_tensor(out=t[:], in0=ag_b[:, 0:NG],
                                        in1=rbcm[:], op=alu.mult)
                elu_into(hTmg[:, b * NG:(b + 1) * NG], t, F(f"molb{b}"), NG)
            gru_batched(4, hTmg, NG, slice(0, NG), NG)

        # ================= MLP head =================
        o1ps = ps_tile("ch", "o1ps")
        for k in range(2):
            nc.tensor.matmul(o1ps[:, 0:NG], lhsT=W("w1T")[:, k * 128:(k + 1) * 128],
                             rhs=outTs[k][:], start=(k == 0), stop=(k == 1))
        o1 = npool.tile([128, NG], BF, tag="o1", name="o1")
        nc.scalar.activation(o1[:], o1ps[:, 0:NG], act.Relu, bias=F("b1"))
        o2ps = ps_tile("gp", "o2ps")
        nc.tensor.matmul(o2ps[0:64, 0:NG], lhsT=W("w2T"), rhs=o1[:],
                         start=True, stop=True)
        o2 = npool.tile([65, NG], BF, tag="o2", name="o2")
        nc.vector.memset(o2[64:65, :], 1.0)
        nc.scalar.activation(o2[0:64, :], o2ps[0:64, 0:NG], act.Relu,
                             bias=F("b2_")[0:64, :])
        o3ps = ps_tile("gp", "o3ps")
        nc.tensor.matmul(o3ps[0:1, 0:NG], lhsT=w3T[:], rhs=o2[:],
                         start=True, stop=True)
        o3 = spool.tile([1, NG], F32, tag="o3", name="o3")
        nc.vector.tensor_copy(o3[:], o3ps[0:1, 0:NG])
        nc.sync.dma_start(y.ap()[:, 0:NG], o3[:])

    return nc


def make_core_inputs(P, inputs, ci, wpb, wpf):
    c = P['cores'][ci]
    NLOC, NWIN, CW = P['NLOC'], P['NWIN'], P['CW']
    x = np.asarray(inputs['x'], np.float32)
    xinT = np.zeros((65, NLOC), np.float32)
    xinT[:64, :c['nn']] = x[c['ns']:c['ne']].T
    xinT[64, :] = 1.0
    ea = np.asarray(inputs['edge_attr'], np.float32)
    ea_perm = np.where(c['sl_fill'][:, None], ea[c['sl_edge']], 0.0)
    eaT2 = ea_perm.reshape(NWIN, CW * 128, 16).transpose(0, 2, 1).astype(BF16)
    return dict(
        xinT=xinT.astype(BF16),
        srcblk=c['srcblk'], dlblk=c['dlblk'],
        selTblk=c['selTblk'], eaT2=eaT2,
        glocT=c['glocT'], selgTblk=c['selgT'],
        wpb=wpb, wpf=wpf,
        lin1T=np.concatenate([np.asarray(inputs['lin1_w'], np.float32).T,
                              np.asarray(inputs['lin1_b'], np.float32)[None, :]],
                             0).astype(BF16),
        W2T=np.asarray(inputs['g_lin1_w'], np.float32)[:, 256:].T.copy().astype(BF16),
        w3T=np.concatenate([np.asarray(inputs['mlp_w3'], np.float32).T,
                            np.asarray(inputs['mlp_b3'], np.float32).reshape(1, 1)],
                           0).astype(BF16),
    )


_CACHE = {}
LAST_EXEC_NS = None

def kernel(**inputs):
    inputs = dict(inputs)
    edge_index = np.asarray(inputs['edge_index']).astype(np.int64)
    batch = np.asarray(inputs['batch']).astype(np.int64)
    n_cores = 8
    G = 2048
    P = preprocess(edge_index, batch, n_cores=n_cores, G=G, CW=5)
    key = (P['NLOC'], P['NWIN'], P['CW'], P['GPC'])
    if key not in _CACHE:
        _CACHE[key] = build_kernel(P['NLOC'], P['NWIN'], P['CW'], P['GPC'],
                                   n_cores)
    nc = _CACHE[key]
    wpb, wpf = make_wpacks(inputs)
    ins = [make_core_inputs(P, inputs, ci, wpb, wpf) for ci in range(n_cores)]
    from concourse.bass_utils import run_bass_kernel_spmd
    trace = bool(os.environ.get('BASS_KERNEL_TRACE'))
    res = run_bass_kernel_spmd(nc, ins, list(range(n_cores)), trace=trace)
    if trace:
        global LAST_EXEC_NS
        LAST_EXEC_NS = res.exec_time_ns
    yv = np.concatenate([np.asarray(res.results[c]['y'][0, :P['GPC']],
                                    np.float32) for c in range(n_cores)])
    return yv.reshape(G, 1).astype(np.float32)
